# revision 19
# baseline (speedup 1.0000x reference)
"""3-layer GAT on 8 Trainium2 NeuronCores (Bass/Tile) — v2.

Strategy vs v1: the huge AllGather of layer-1 features (115MB) is gone.
Host replicates the (block-mapped) raw x table to every core; layer 1
gathers 512B x rows per edge and transforms them on the PE fused with the
selection-matmul aggregation. A tiny [s1|d1] logit table is all-gathered
in 4 row-block collectives. Layers 2/3 keep the table-gather scheme, but
their AllGathers are split into 4 row-block collectives that fire as the
producing tiles complete, overlapping the previous layer's compute.

Self-contained: only imports the system concourse install.
"""

import os
import sys

for _p in ("/opt/trn_rl_repo", "/root/.axon_site/_ro/trn_rl_repo"):
    if os.path.isdir(_p) and _p not in sys.path:
        sys.path.insert(0, _p)

import math
from dataclasses import dataclass

import ml_dtypes
import numpy as np

import concourse.bacc as bacc
import concourse.bass as bass
import concourse.tile as tile
from concourse import mybir
from concourse.bass_utils import run_bass_kernel_spmd

P = 128
BF16 = mybir.dt.bfloat16
F32 = mybir.dt.float32
I16 = mybir.dt.int16
AL = mybir.AluOpType
AF = mybir.ActivationFunctionType

NEG_SLOPE_ATT = 0.2
NEG_SLOPE_ACT = 0.01
LN_EPS = 1e-5


def _ceil(a, b):
    return -(-a // b)


def _ceil_arr(a, b):
    return -(-a // b)


def _pad_elem(n_f32_elems):
    """bf16 row length (elements) padded so row bytes are a multiple of 256."""
    return _ceil(n_f32_elems * 2, 256) * 128


@dataclass
class Cfg:
    N: int = 50000
    E: int = 400000
    F_IN: int = 256
    HEADS: int = 4
    C1: int = 256
    C2: int = 128
    NCLS: int = 32
    NCORES: int = 8

    def __post_init__(self):
        assert self.N % self.NCORES == 0
        self.NL = self.N // self.NCORES
        self.T = _ceil(self.NL, P)
        self.NLP = self.T * P
        self.NPTOT = self.NLP * self.NCORES
        # 4 row blocks of whole tiles; AllGathers fire per block, edge
        # chunks are grouped by source block (gather tables are per-block
        # Shared tensors — the Tile checker allows one writer per tensor).
        self.NG = 4
        self.BLK_T = [13, 12, 12, 12]
        assert sum(self.BLK_T) == self.T
        self.SBT = [0]
        for b in self.BLK_T[:-1]:
            self.SBT.append(self.SBT[-1] + b)
        self.GROWS = [b * P * self.NCORES for b in self.BLK_T]
        self.GBASE = [0]
        for b in self.GROWS[:-1]:
            self.GBASE.append(self.GBASE[-1] + b)
        assert max(self.GROWS) < 32768  # int16 gather indices per group
        H = self.HEADS
        self.CO1 = H * self.C1
        self.CO2 = H * self.C2
        assert self.F_IN % P == 0 and self.CO1 % P == 0 and self.CO2 % P == 0
        self.ELEM2 = _pad_elem(self.CO2 + H)          # [h2|s2|pad] rows
        self.ELEM3 = _pad_elem(self.NCLS + 1)         # [h3|s3|pad] rows
        self.W1w = self.CO1 + 2 * H                   # [W1 | U_s | U_d]
        self.W2w = self.CO2 + 2 * H
        self.W3w = self.NCLS + 2


@dataclass
class Meta:
    nch: list  # [T][NG] chunk counts (common across cores)
    si: list   # [T][NG] idx16 column offsets
    sc: list   # [T][NG] dstloc column offsets
    SI: int
    SC: int
    sd: list   # [T] dst-idx column offsets (per-tile d gather)
    SD: int


def _grp_map(cfg: Cfg, core, loc):
    """(source block, within-block row index) for node (core, local idx)."""
    t = loc // P
    b = np.searchsorted(np.array(cfg.SBT + [cfg.T]), t, side="right") - 1
    blk_t = np.array(cfg.BLK_T)[b]
    sb = np.array(cfg.SBT)[b]
    return b, core * blk_t * P + (loc - sb * P)


def _gidx_map(cfg: Cfg, core, loc):
    """Global row index in the concatenated block-mapped table."""
    b, off = _grp_map(cfg, core, loc)
    return np.array(cfg.GBASE)[b] + off


def host_prep(cfg: Cfg, x, edge_src, edge_dst,
              W1, a_src1, a_dst1, b1, ln1_g, ln1_b,
              W2, a_src2, a_dst2, b2, ln2_g, ln2_b,
              W3, a_src3, a_dst3, b3, ln3_g, ln3_b):
    c = cfg
    bf = ml_dtypes.bfloat16

    # ---- append self loops, shard edges by destination core
    loops = np.arange(c.N, dtype=np.int64)
    src = np.concatenate([edge_src.astype(np.int64), loops])
    dst = np.concatenate([edge_dst.astype(np.int64), loops])

    dst_core = dst // c.NL
    dstloc = dst - dst_core * c.NL
    tile_id = dstloc // P
    grp, idx16 = _grp_map(c, src // c.NL, src % c.NL)
    grp = grp.astype(np.int64)
    idx16 = idx16.astype(np.int64)
    NG = c.NG

    counts = np.zeros((c.NCORES, c.T, NG), np.int64)
    np.add.at(counts, (dst_core, tile_id, grp), 1)
    nch = np.maximum(_ceil_arr(counts.max(axis=0), P), 0)  # [T,NG] chunks
    si = np.zeros((c.T, NG), np.int64)
    sc = np.zeros((c.T, NG), np.int64)
    acc_si = acc_sc = 0
    for t in range(c.T):
        for h in range(NG):
            si[t, h] = acc_si
            sc[t, h] = acc_sc
            acc_si += int(nch[t, h]) * (P // 16)
            acc_sc += int(nch[t, h])
    SI, SC = int(acc_si), int(acc_sc)
    sd = np.zeros(c.T, np.int64)
    acc_sd = 0
    for t in range(c.T):
        sd[t] = acc_sd
        acc_sd += int(sum(nch[t])) * (P // 16)
    SD = int(acc_sd)
    meta = Meta(nch=nch.tolist(), si=si.tolist(), sc=sc.tolist(), SI=SI,
                SC=SC, sd=sd.tolist(), SD=SD)

    order = np.lexsort((grp, tile_id, dst_core))
    src_s = idx16[order]
    dstrel_s = (dstloc - tile_id * P)[order]

    starts = np.zeros((c.NCORES, c.T, NG), np.int64)
    run = 0
    for cc in range(c.NCORES):
        for t in range(c.T):
            for h in range(NG):
                starts[cc, t, h] = run
                run += int(counts[cc, t, h])

    idx_tabs, dl_tabs, dstidx_tabs = [], [], []
    for cc in range(c.NCORES):
        itab = np.zeros((16, SI), np.int16)
        dtab = np.full((P, SC), -1.0, np.float32)
        ditab = np.zeros((16, SD), np.int16)
        for t in range(c.T):
            dchunk = 0
            for h in range(NG):
                m = int(counts[cc, t, h])
                n = int(nch[t, h])
                if n == 0:
                    continue
                s0 = int(starts[cc, t, h])
                iv = np.zeros(n * P, np.int16)
                iv[:m] = src_s[s0:s0 + m].astype(np.int16)
                cols = int(si[t, h])
                itab[:, cols:cols + n * (P // 16)] = iv.reshape(
                    n * P // 16, 16).T
                dv = np.full(n * P, -1.0, np.float32)
                dv[:m] = dstrel_s[s0:s0 + m].astype(np.float32)
                dtab[:, sc[t, h]:sc[t, h] + n] = dv.reshape(n, P).T
                div = np.zeros(n * P, np.int16)
                div[:m] = (t * P + dstrel_s[s0:s0 + m]).astype(np.int16)
                dc = int(sd[t]) + dchunk * (P // 16)
                ditab[:, dc:dc + n * (P // 16)] = div.reshape(
                    n * P // 16, 16).T
                dchunk += n
        idx_tabs.append(np.tile(itab, (8, 1)))
        dl_tabs.append(dtab)
        dstidx_tabs.append(np.tile(ditab, (8, 1)))

    # ---- block-mapped full x table (replicated to every core)
    xfull = np.zeros((c.NPTOT, c.F_IN), np.float32)
    for cc in range(c.NCORES):
        loc = np.arange(c.NL)
        gi = _gidx_map(c, np.full(c.NL, cc), loc)
        xfull[gi] = x[cc * c.NL:(cc + 1) * c.NL]
    xfull = xfull.astype(bf)

    # ---- weights (augmented with U = W.T @ a columns), bf16
    def aug(W, a_s, a_d, H, C):
        WT = W.T.astype(np.float64)
        U_s = np.zeros((WT.shape[0], H))
        U_d = np.zeros((WT.shape[0], H))
        for h in range(H):
            U_s[:, h] = WT[:, h * C:(h + 1) * C] @ a_s[h].astype(np.float64)
            U_d[:, h] = WT[:, h * C:(h + 1) * C] @ a_d[h].astype(np.float64)
        return np.concatenate([WT, U_s, U_d], axis=1).astype(bf)

    W1a = aug(W1, a_src1, a_dst1, c.HEADS, c.C1)   # [F_IN, CO1+2H]
    W2a = aug(W2, a_src2, a_dst2, c.HEADS, c.C2)   # [CO1, CO2+2H]
    W3a = aug(W3, a_src3, a_dst3, 1, c.NCLS)       # [CO2, NCLS+2]

    def bln(b, g, be):
        row = np.concatenate([b, g, be]).astype(np.float32)[None, :]
        return np.repeat(row, P, axis=0)

    bln1 = bln(b1, ln1_g, ln1_b)
    bln2 = bln(b2, ln2_g, ln2_b)
    bln3 = bln(b3, ln3_g, ln3_b)

    ident = np.eye(P, dtype=bf)
    iota_f = np.repeat(np.arange(P, dtype=np.float32)[None, :], P, axis=0)

    in_maps = []
    for cc in range(c.NCORES):
        xl = np.zeros((c.NLP, c.F_IN), np.float32)
        xl[:c.NL] = x[cc * c.NL:(cc + 1) * c.NL]
        in_maps.append({
            "xT": np.ascontiguousarray(xl.T).astype(bf),
            "xfull": xfull,
            "W1a": W1a, "W2a": W2a, "W3a": W3a,
            "bln1": bln1, "bln2": bln2, "bln3": bln3,
            "idx16": idx_tabs[cc], "dstloc": dl_tabs[cc],
            "dstidx16": dstidx_tabs[cc],
            "iotaf": iota_f, "ident": ident,
        })
    return in_maps, meta


# --------------------------------------------------------------------------
# device program
# --------------------------------------------------------------------------

def build_nc(cfg: Cfg, meta: Meta):
    c = cfg
    H = c.HEADS
    nc = bacc.Bacc("TRN2", target_bir_lowering=False, debug=False,
                   num_devices=c.NCORES, enable_partition_id=False)

    # ---- I/O
    xT = nc.dram_tensor("xT", [c.F_IN, c.NLP], BF16, kind="ExternalInput").ap()
    xfull = nc.dram_tensor("xfull", [c.NPTOT, c.F_IN], BF16,
                           kind="ExternalInput").ap()
    W1a = nc.dram_tensor("W1a", [c.F_IN, c.W1w], BF16, kind="ExternalInput").ap()
    W2a = nc.dram_tensor("W2a", [c.CO1, c.W2w], BF16, kind="ExternalInput").ap()
    W3a = nc.dram_tensor("W3a", [c.CO2, c.W3w], BF16, kind="ExternalInput").ap()
    bln1 = nc.dram_tensor("bln1", [P, 3 * c.CO1], F32, kind="ExternalInput").ap()
    bln2 = nc.dram_tensor("bln2", [P, 3 * c.CO2], F32, kind="ExternalInput").ap()
    bln3 = nc.dram_tensor("bln3", [P, 3 * c.NCLS], F32, kind="ExternalInput").ap()
    idx16 = nc.dram_tensor("idx16", [P, meta.SI], I16, kind="ExternalInput").ap()
    dstloc = nc.dram_tensor("dstloc", [P, meta.SC], F32, kind="ExternalInput").ap()
    dstidx16 = nc.dram_tensor("dstidx16", [P, meta.SD], I16,
                              kind="ExternalInput").ap()
    iotaf = nc.dram_tensor("iotaf", [P, P], F32, kind="ExternalInput").ap()
    ident = nc.dram_tensor("ident", [P, P], BF16, kind="ExternalInput").ap()
    y = nc.dram_tensor("y", [c.NLP, c.NCLS], F32, kind="ExternalOutput").ap()

    groups = [list(range(c.NCORES))]

    def block_ag(loc_t, full_ts):
        """Emit the per-block AllGathers loc -> full_ts[b] (block-mapped)."""
        for b in range(c.NG):
            r0 = c.SBT[b] * P
            nr = c.BLK_T[b] * P
            nc.gpsimd.collective_compute(
                "AllGather", AL.bypass, replica_groups=groups,
                ins=[loc_t[r0:r0 + nr, :].opt()],
                outs=[full_ts[b][:].opt()])

    with tile.TileContext(nc) as tc:
        dram_cm = tc.tile_pool(name="dram", bufs=1, space="DRAM")
        dram = dram_cm.__enter__()
        ds1_loc = dram.tile([c.NLP, P], BF16)               # [s1|d1|pad] rows
        ds1_full = [dram.tile([c.GROWS[b], P], BF16, addr_space="Shared",
                          name=f"ds1f{b}") for b in range(c.NG)]
        hs2_loc = dram.tile([c.NLP, c.ELEM2], BF16)
        hs2_full = [dram.tile([c.GROWS[b], c.ELEM2], BF16, addr_space="Shared",
                          name=f"hs2f{b}") for b in range(c.NG)]
        hs3_loc = dram.tile([c.NLP, c.ELEM3], BF16)
        hs3_full = [dram.tile([c.GROWS[b], c.ELEM3], BF16, addr_space="Shared",
                          name=f"hs3f{b}") for b in range(c.NG)]
        d2t = dram.tile([c.NLP, P], BF16)
        d3t = dram.tile([c.NLP, P], BF16)

        # ---- persistent SBUF constants
        cpool_cm = tc.tile_pool(name="const", bufs=1)
        cpool = cpool_cm.__enter__()
        KC1 = c.F_IN // P
        W1a_sb = cpool.tile([P, KC1 * c.W1w], BF16)
        for k in range(KC1):
            nc.sync.dma_start(W1a_sb[:, k * c.W1w:(k + 1) * c.W1w],
                              W1a[k * P:(k + 1) * P, :])
        KC2 = c.CO1 // P
        W2a_sb = cpool.tile([P, KC2 * c.W2w], BF16)
        for k in range(KC2):
            nc.sync.dma_start(W2a_sb[:, k * c.W2w:(k + 1) * c.W2w],
                              W2a[k * P:(k + 1) * P, :])
        KC3 = c.CO2 // P
        W3a_sb = cpool.tile([P, KC3 * c.W3w], BF16)
        for k in range(KC3):
            nc.sync.dma_start(W3a_sb[:, k * c.W3w:(k + 1) * c.W3w],
                              W3a[k * P:(k + 1) * P, :])
        bln1_sb = cpool.tile([P, 3 * c.CO1], F32)
        nc.sync.dma_start(bln1_sb[:], bln1[:])
        bln2_sb = cpool.tile([P, 3 * c.CO2], F32)
        nc.sync.dma_start(bln2_sb[:], bln2[:])
        bln3_sb = cpool.tile([P, 3 * c.NCLS], F32)
        nc.sync.dma_start(bln3_sb[:], bln3[:])
        idx_sb = cpool.tile([P, meta.SI], I16)
        nc.sync.dma_start(idx_sb[:], idx16[:])
        dl_sb = cpool.tile([P, meta.SC], F32)
        nc.sync.dma_start(dl_sb[:], dstloc[:])
        didx_sb = cpool.tile([P, meta.SD], I16)
        nc.sync.dma_start(didx_sb[:], dstidx16[:])
        iota_sb = cpool.tile([P, P], F32)
        nc.sync.dma_start(iota_sb[:], iotaf[:])
        id_sb = cpool.tile([P, P], BF16)
        nc.sync.dma_start(id_sb[:], ident[:])

        # ================= prologue: [s1|d1] logit table + 4-block AG
        with (
            tc.tile_pool(name="pro", bufs=3) as pro,
            tc.tile_pool(name="prop", bufs=2, space="PSUM") as prop,
        ):
            for t in range(c.T):
                xt = pro.tile([P, KC1 * P], BF16, tag="xt")
                for k in range(KC1):
                    nc.sync.dma_start(xt[:, k * P:(k + 1) * P],
                                      xT[k * P:(k + 1) * P, t * P:(t + 1) * P])
                dsp = prop.tile([P, 2 * H], F32, tag="dsp")
                for k in range(KC1):
                    nc.tensor.matmul(
                        out=dsp[:],
                        lhsT=xt[:, k * P:(k + 1) * P],
                        rhs=W1a_sb[:, k * c.W1w + c.CO1:k * c.W1w + c.CO1 + 2 * H],
                        start=(k == 0), stop=(k == KC1 - 1))
                dsb = pro.tile([P, P], BF16, tag="dsb")
                nc.vector.tensor_copy(dsb[:, 0:2 * H], dsp[:])
                nc.vector.memset(dsb[:, 2 * H:P], 0)
                nc.sync.dma_start(ds1_loc[t * P:(t + 1) * P, :], dsb[:])
        block_ag(ds1_loc, ds1_full)

        # ================= layer 1: gather x rows, transform + aggregate
        _l1_phase(nc, tc, c, meta, xfull, ds1_loc, ds1_full, W1a_sb, W2a_sb,
                  bln1_sb, iota_sb, id_sb, idx_sb, dl_sb, didx_sb,
                  hs2_loc, d2t)
        block_ag(hs2_loc, hs2_full)

        # ================= layer 2 (+fused L3 transform)
        _edge_phase(
            nc, tc, c, meta, lay=2, Hn=H, Ch=c.C2, ELEM=c.ELEM2,
            hs_full=hs2_full, d_tab=d2t, bln_sb=bln2_sb,
            iota_sb=iota_sb, id_sb=id_sb, idx_sb=idx_sb, dl_sb=dl_sb,
            didx_sb=didx_sb,
            fuse=dict(W_sb=W3a_sb, KC=KC3, Ww=c.W3w, CO=c.NCLS, Hn2=1,
                      ELEMn=c.ELEM3, hs_loc=hs3_loc, d_next=d3t),
            final=None, y=None)
        block_ag(hs3_loc, hs3_full)

        # ================= layer 3 + log_softmax
        _edge_phase(
            nc, tc, c, meta, lay=3, Hn=1, Ch=c.NCLS, ELEM=c.ELEM3,
            hs_full=hs3_full, d_tab=d3t, bln_sb=bln3_sb,
            iota_sb=iota_sb, id_sb=id_sb, idx_sb=idx_sb, dl_sb=dl_sb,
            didx_sb=didx_sb,
            fuse=None, final=True, y=y)

        cpool_cm.__exit__(None, None, None)
        dram_cm.__exit__(None, None, None)

    nc.compile()
    return nc


# --------------------------------------------------------------------------
# layer-1 phase: per-edge transform fused with aggregation
# --------------------------------------------------------------------------

def _l1_phase(nc, tc, c: Cfg, meta: Meta, xfull, ds1_loc, ds1_full,
              W1a_sb, W2a_sb, bln_sb, iota_sb, id_sb, idx_sb, dl_sb, didx_sb,
              hs2_loc, d2t):
    H = c.HEADS
    CO = c.CO1
    Ch = c.C1
    max_nch = max(max(r) for r in meta.nch)
    max_ntot = max(sum(r) for r in meta.nch)
    KC1 = c.F_IN // P
    GH_ACT = 640          # PSUM->SBUF copy split: ACT cols, DVE the rest

    with (
        tc.tile_pool(name="gx", bufs=2) as gxp,
        tc.tile_pool(name="sg", bufs=2) as sgp,
        tc.tile_pool(name="gh", bufs=2 * max_ntot + 2) as ghp,
        tc.tile_pool(name="w1", bufs=2) as wp,
        tc.tile_pool(name="s1", bufs=3) as sp,
        tc.tile_pool(name="ep", bufs=1) as ep,
        tc.tile_pool(name="x2p", bufs=2) as x2p,
        tc.tile_pool(name="hst1", bufs=2) as hstp,
        tc.tile_pool(name="php", bufs=2, space="PSUM") as php,
        tc.tile_pool(name="pagg", bufs=1, space="PSUM") as pagg,
        tc.tile_pool(name="pden", bufs=1, space="PSUM") as pden,
        tc.tile_pool(name="pscr", bufs=1, space="PSUM") as pscr,
    ):
        state = {}  # per-tile live tiles: Gx, Sg, Dg, Gh list, agg, den, x2

        def stage_gather(t):
            nchs = meta.nch[t]
            st = state.setdefault(t, {})
            # d rows (tile-local) from ds1_loc, all groups' chunks
            Dg = sgp.tile([P, max_ntot * P], BF16, tag="Dg")
            b0 = 0
            for g in range(c.NG):
                nch = nchs[g]
                if nch == 0:
                    continue
                sdo = meta.sd[t] + b0 * (P // 16)
                nc.gpsimd.dma_gather(
                    out_ap=Dg[:, b0 * P:(b0 + nch) * P].rearrange(
                        "p (k d) -> p k d", d=P),
                    in_ap=ds1_loc[:],
                    idxs_ap=didx_sb[:, sdo:sdo + nch * (P // 16)],
                    num_idxs=nch * P, num_idxs_reg=nch * P, elem_size=P)
                b0 += nch
            st["Dg"] = Dg
            Gxs, Sgs = [], []
            for g in range(c.NG):
                nch = nchs[g]
                if nch == 0:
                    Gxs.append(None)
                    Sgs.append(None)
                    continue
                si = meta.si[t][g]
                nidx = nch * P
                base = c.GBASE[g]
                # s rows (per-block table)
                Sg = sgp.tile([P, max_nch * P], BF16, tag=f"Sg{g}")
                nc.gpsimd.dma_gather(
                    out_ap=Sg[:, 0:nch * P].rearrange("p (k d) -> p k d", d=P),
                    in_ap=ds1_full[g][:],
                    idxs_ap=idx_sb[:, si:si + nch * (P // 16)],
                    num_idxs=nidx, num_idxs_reg=nidx, elem_size=P)
                Sgs.append(Sg)
                # x rows (block slice of xfull), transpose mode: [128ch, j, e]
                Gx = gxp.tile([P, KC1 * max_nch * P], BF16, tag=f"Gx{g}")
                nc.gpsimd.dma_gather(
                    out_ap=Gx[:, 0:KC1 * nidx].rearrange(
                        "p (j e) -> p j e", e=nidx),
                    in_ap=xfull[base:base + c.GROWS[g], :],
                    idxs_ap=idx_sb[:, si:si + nch * (P // 16)],
                    num_idxs=nidx, num_idxs_reg=nidx, elem_size=c.F_IN,
                    transpose=True)
                Gxs.append(Gx)
            st["Gx"] = Gxs
            st["Sg"] = Sgs

        def stage_transform(t):
            nchs = meta.nch[t]
            st = state[t]
            Ghs = []
            for g in range(c.NG):
                Gx = st["Gx"][g]
                if Gx is None:
                    continue
                nch = nchs[g]
                nidx = nch * P
                Gxv = Gx[:, 0:KC1 * nidx].rearrange("p (j e) -> p j e", e=nidx)
                for ck in range(nch):
                    hp = php.tile([P, CO], F32, tag="hp")
                    for j in range(KC1):
                        for n0 in range(0, CO, 512):
                            nc.tensor.matmul(
                                out=hp[:, n0:n0 + 512],
                                lhsT=Gxv[:, j, ck * P:(ck + 1) * P],
                                rhs=W1a_sb[:, j * c.W1w + n0:j * c.W1w + n0 + 512],
                                start=(j == 0), stop=(j == KC1 - 1))
                    Gh = ghp.tile([P, CO], BF16, tag="gh")
                    nc.scalar.copy(Gh[:, 0:GH_ACT], hp[:, 0:GH_ACT])
                    nc.vector.tensor_copy(Gh[:, GH_ACT:CO], hp[:, GH_ACT:CO])
                    Ghs.append(Gh)
            st["Gh"] = Ghs

        def stage_w_agg(t):
            nchs = meta.nch[t]
            ntot = sum(nchs)
            st = state[t]
            agg = pagg.tile([P, CO], F32, tag="agg")
            den = pden.tile([P, H], F32, tag="den")
            st["agg"], st["den"] = agg, den
            if ntot == 0:
                nc.vector.memset(agg[:], 0)
                nc.vector.memset(den[:], 0)
                return
            # batched w chain
            eqa = wp.tile([P, max_ntot * P], BF16, tag="eqa")
            c0 = meta.sc[t][0]
            dlv = dl_sb[:, c0:c0 + ntot].to_broadcast([P, ntot, P])
            io = iota_sb[:]
            iob = bass.AP(io.tensor, io.offset,
                          [list(io.ap[0]), [0, ntot], list(io.ap[1])])
            nc.vector.tensor_tensor(
                out=eqa[:, 0:ntot * P].rearrange("p (k d) -> p k d", d=P),
                in0=dlv, in1=iob, op=AL.is_equal)
            tsda = wp.tile([P, max_ntot * H], F32, tag="tsda")
            b0 = 0
            for g in range(c.NG):
                nch = nchs[g]
                if nch == 0:
                    continue
                Sgv = st["Sg"][g][:, 0:nch * P].rearrange(
                    "p (k d) -> p k d", d=P)[:, :, 0:H]
                Dv = st["Dg"][:, b0 * P:(b0 + nch) * P].rearrange(
                    "p (k d) -> p k d", d=P)[:, :, H:2 * H]
                nc.vector.tensor_tensor(
                    out=tsda[:, b0 * H:(b0 + nch) * H].rearrange(
                        "p (k h) -> p k h", h=H),
                    in0=Sgv, in1=Dv, op=AL.add)
                b0 += nch
            lra = wp.tile([P, max_ntot * H], F32, tag="lra")
            nc.vector.scalar_tensor_tensor(
                out=lra[:, 0:ntot * H], in0=tsda[:, 0:ntot * H],
                scalar=NEG_SLOPE_ATT, in1=tsda[:, 0:ntot * H],
                op0=AL.mult, op1=AL.max)
            wfa = wp.tile([P, max_ntot * H], F32, tag="wfa")
            nc.scalar.activation(wfa[:, 0:ntot * H], lra[:, 0:ntot * H], AF.Exp)
            wfb = wp.tile([P, max_ntot * H], BF16, tag="wfb")
            nc.vector.tensor_copy(wfb[:, 0:ntot * H], wfa[:, 0:ntot * H])

            for k in range(ntot):
                first, last = (k == 0), (k == ntot - 1)
                eq = eqa[:, k * P:(k + 1) * P]
                wf = wfa[:, k * H:(k + 1) * H]
                S = sp.tile([P, H * P], BF16, tag="S")
                eq_b = bass.AP(eq.tensor, eq.offset,
                               [list(eq.ap[0]), [0, H], list(eq.ap[1])])
                nc.vector.tensor_tensor(
                    out=S[:].rearrange("p (h d) -> p h d", h=H),
                    in0=eq_b, in1=wf.to_broadcast([P, H, P]), op=AL.mult)
                Gh = st["Gh"][k]
                for h in range(H):
                    nc.tensor.matmul(
                        out=agg[:, h * Ch:(h + 1) * Ch],
                        lhsT=S[:, h * P:(h + 1) * P],
                        rhs=Gh[:, h * Ch:(h + 1) * Ch],
                        start=first and (h * Ch) % 512 == 0,
                        stop=last and ((h + 1) * Ch) % 512 == 0)
                nc.tensor.matmul(out=den[:], lhsT=eq,
                                 rhs=wfb[:, k * H:(k + 1) * H],
                                 start=first, stop=last)

        def stage_ln(t):
            """agg/den -> x2 (leaky(LN(agg/den + bias))), lean engine split."""
            st = state[t]
            agg, den = st["agg"], st["den"]
            denr = ep.tile([P, H], F32, tag="denr")
            nc.vector.tensor_scalar(out=denr[:], in0=den[:], scalar1=1e-16,
                                    scalar2=None, op0=AL.add)
            rec = ep.tile([P, H], F32, tag="rec")
            nc.vector.reciprocal(rec[:], denr[:])
            ob = ep.tile([P, CO], BF16, tag="ob")
            for h in range(H):
                nc.vector.scalar_tensor_tensor(
                    out=ob[:, h * Ch:(h + 1) * Ch],
                    in0=agg[:, h * Ch:(h + 1) * Ch],
                    scalar=rec[:, h:h + 1], op0=AL.mult,
                    in1=bln_sb[:, h * Ch:(h + 1) * Ch], op1=AL.add)
            rs = ep.tile([P, 1], F32, tag="rs")
            nc.vector.tensor_reduce(out=rs[:], in_=ob[:],
                                    axis=mybir.AxisListType.X, op=AL.add)
            nm = ep.tile([P, 1], F32, tag="nm")
            nc.vector.tensor_scalar(out=nm[:], in0=rs[:], scalar1=-1.0 / CO,
                                    scalar2=None, op0=AL.mult)
            xc = ep.tile([P, CO], BF16, tag="xc")
            nc.scalar.activation(xc[:], ob[:], AF.Identity, bias=nm[:, 0:1])
            sqd = ep.tile([P, CO], BF16, tag="sqd")
            vs = ep.tile([P, 1], F32, tag="vs")
            nc.scalar.activation(sqd[:], xc[:], AF.Square, accum_out=vs[:])
            vstd = ep.tile([P, 1], F32, tag="vstd")
            nc.vector.tensor_scalar(out=vstd[:], in0=vs[:], scalar1=1.0 / CO,
                                    scalar2=LN_EPS, op0=AL.mult, op1=AL.add)
            sd = ep.tile([P, 1], F32, tag="sd")
            nc.scalar.activation(sd[:], vstd[:], AF.Sqrt)
            rstd = ep.tile([P, 1], F32, tag="rstd")
            nc.vector.reciprocal(rstd[:], sd[:])
            y1 = ep.tile([P, CO], BF16, tag="y1")
            nc.vector.scalar_tensor_tensor(
                out=y1[:], in0=xc[:], scalar=rstd[:, 0:1],
                in1=bln_sb[:, CO:2 * CO], op0=AL.mult, op1=AL.mult)
            y2 = ep.tile([P, CO], BF16, tag="y2")
            nc.vector.tensor_tensor(out=y2[:], in0=y1[:],
                                    in1=bln_sb[:, 2 * CO:3 * CO], op=AL.add)
            x2 = x2p.tile([P, CO], BF16, tag="x2")
            nc.vector.scalar_tensor_tensor(
                out=x2[:], in0=y2[:], scalar=NEG_SLOPE_ACT, in1=y2[:],
                op0=AL.mult, op1=AL.max)
            st["x2"] = x2

        def stage_tail(t):
            """x2 -> transposed -> fused L2 transform -> hs2/d2 stores."""
            st = state[t]
            x2 = st["x2"]
            KCn = CO // P
            xt2 = ep.tile([P, KCn * P], BF16, tag="xt2")
            for k in range(KCn):
                scr = pscr.tile([P, P], BF16, tag="scr")
                nc.tensor.transpose(out=scr[:], in_=x2[:, k * P:(k + 1) * P],
                                    identity=id_sb[:])
                nc.scalar.copy(xt2[:, k * P:(k + 1) * P], scr[:])
            hp = php.tile([P, CO], F32, tag="hp")
            _mm_splits(nc, hp[:, 0:c.W2w], xt2, W2a_sb, KCn, c.W2w, P)
            _store_hs(nc, hstp, hp, c.CO2, c.HEADS, c.ELEM2, hs2_loc, d2t, t)
            del state[t]

        # software-pipelined emission: PE never waits on the LN chain
        for i in range(c.T + 2):
            if i < c.T:
                stage_gather(i)
                stage_transform(i)
                stage_w_agg(i)
                stage_ln(i)
            if i - 2 >= 0:
                stage_tail(i - 2)


def _mm_splits(nc, out_ps, lhs_sb, w_sb, KC, Ww, Plhs):
    """out_ps[:, :Ww] = sum_k lhs_k.T @ W_k, with N split at 512."""
    splits = []
    n0 = 0
    while n0 < Ww:
        nsz = min(512, Ww - n0)
        splits.append((n0, nsz))
        n0 += nsz
    for k in range(KC):
        for (n0, nsz) in splits:
            nc.tensor.matmul(
                out=out_ps[:, n0:n0 + nsz],
                lhsT=lhs_sb[:, k * Plhs:(k + 1) * Plhs],
                rhs=w_sb[:, k * Ww + n0:k * Ww + n0 + nsz],
                start=(k == 0), stop=(k == KC - 1))


def _store_hs(nc, pool, hp, CO, Hn, ELEM, hs_loc, d_tab, t):
    """PSUM [128, CO+2H] -> bf16 hs row tile + bf16 d table row tile."""
    hst = pool.tile([P, ELEM], BF16, tag="hst")
    nc.scalar.copy(hst[:, 0:CO], hp[:, 0:CO])
    nc.vector.tensor_copy(hst[:, CO:CO + Hn], hp[:, CO:CO + Hn])
    if ELEM > CO + Hn:
        nc.vector.memset(hst[:, CO + Hn:ELEM], 0)
    dt = pool.tile([P, P], BF16, tag="dt")
    nc.vector.tensor_copy(dt[:, 0:Hn], hp[:, CO + Hn:CO + 2 * Hn])
    nc.vector.memset(dt[:, Hn:P], 0)
    nc.sync.dma_start(hs_loc[t * P:(t + 1) * P, :], hst[:])
    nc.sync.dma_start(d_tab[t * P:(t + 1) * P, :], dt[:])


# --------------------------------------------------------------------------
# layers 2/3: table-gather edge phase (baseline machinery, block halves)
# --------------------------------------------------------------------------

def _edge_phase(nc, tc, c: Cfg, meta: Meta, lay, Hn, Ch, ELEM, hs_full, d_tab,
                bln_sb, iota_sb, id_sb, idx_sb, dl_sb, didx_sb,
                fuse, final, y):
    CO = Hn * Ch
    max_nch = max(max(r) for r in meta.nch)
    max_ntot = max(sum(r) for r in meta.nch)
    merge_den = (Hn == 1)

    with (
        tc.tile_pool(name=f"sb{lay}", bufs=2) as sb,
        tc.tile_pool(name=f"sc{lay}", bufs=4) as sbc,
        tc.tile_pool(name=f"g{lay}", bufs=2) as gp,
        tc.tile_pool(name=f"ps{lay}", bufs=1, space="PSUM") as ps1,
        tc.tile_pool(name=f"psagg{lay}", bufs=2, space="PSUM") as psA,
    ):
        for t in range(c.T):
            nchs = meta.nch[t]
            ntot = sum(nchs)
            agg = psA.tile([P, CO + (1 if merge_den else 0)], F32, tag="agg")
            if merge_den:
                den_ap = agg[:, CO:CO + 1]
            else:
                den_t = ps1.tile([P, Hn], F32, tag="den")
                den_ap = den_t[:]
            if ntot == 0:
                nc.vector.memset(agg[:], 0)
                if not merge_den:
                    nc.vector.memset(den_t[:], 0)
                _epilogue(nc, sb, ps1, c, meta, lay, t, agg, den_ap, Hn, Ch,
                          CO, bln_sb, id_sb, fuse, final, y)
                continue

            Dg = gp.tile([P, max_ntot * P], BF16, tag="Dg")
            b0 = 0
            for g in range(c.NG):
                nch = nchs[g]
                if nch == 0:
                    continue
                sdo = meta.sd[t] + b0 * (P // 16)
                nc.gpsimd.dma_gather(
                    out_ap=Dg[:, b0 * P:(b0 + nch) * P].rearrange(
                        "p (k d) -> p k d", d=P),
                    in_ap=d_tab[:],
                    idxs_ap=didx_sb[:, sdo:sdo + nch * (P // 16)],
                    num_idxs=nch * P, num_idxs_reg=nch * P, elem_size=P)
                b0 += nch

            Gs = []
            for g in range(c.NG):
                nch = nchs[g]
                if nch == 0:
                    Gs.append(None)
                    continue
                G = gp.tile([P, max_nch * ELEM], BF16, tag=f"G{g}")
                si = meta.si[t][g]
                nidx = nch * P
                nc.gpsimd.dma_gather(
                    out_ap=G[:, 0:nch * ELEM].rearrange(
                        "p (k d) -> p k d", d=ELEM),
                    in_ap=hs_full[g][:],
                    idxs_ap=idx_sb[:, si:si + nch * (P // 16)],
                    num_idxs=nidx, num_idxs_reg=nidx, elem_size=ELEM)
                Gs.append(G)

            eqa = sbc.tile([P, max_ntot * P], BF16, tag="eqa")
            c0 = meta.sc[t][0]
            dlv = dl_sb[:, c0:c0 + ntot].to_broadcast([P, ntot, P])
            io = iota_sb[:]
            iob = bass.AP(io.tensor, io.offset,
                          [list(io.ap[0]), [0, ntot], list(io.ap[1])])
            nc.vector.tensor_tensor(
                out=eqa[:, 0:ntot * P].rearrange("p (k d) -> p k d", d=P),
                in0=dlv, in1=iob, op=AL.is_equal)
            tsda = sbc.tile([P, max_ntot * Hn], F32, tag="tsda")
            b0 = 0
            for g in range(c.NG):
                nch = nchs[g]
                if nch == 0:
                    continue
                Gv = Gs[g][:, 0:nch * ELEM].rearrange(
                    "p (k d) -> p k d", d=ELEM)[:, :, CO:CO + Hn]
                Dv = Dg[:, b0 * P:(b0 + nch) * P].rearrange(
                    "p (k d) -> p k d", d=P)[:, :, 0:Hn]
                nc.vector.tensor_tensor(
                    out=tsda[:, b0 * Hn:(b0 + nch) * Hn].rearrange(
                        "p (k h) -> p k h", h=Hn),
                    in0=Gv, in1=Dv, op=AL.add)
                b0 += nch
            lra = sbc.tile([P, max_ntot * Hn], F32, tag="lra")
            nc.vector.scalar_tensor_tensor(
                out=lra[:, 0:ntot * Hn], in0=tsda[:, 0:ntot * Hn],
                scalar=NEG_SLOPE_ATT, in1=tsda[:, 0:ntot * Hn],
                op0=AL.mult, op1=AL.max)
            wfa = sbc.tile([P, max_ntot * Hn], F32, tag="wfa")
            nc.scalar.activation(wfa[:, 0:ntot * Hn], lra[:, 0:ntot * Hn],
                                 AF.Exp)
            wfb = sbc.tile([P, max_ntot * Hn], BF16, tag="wfb")
            nc.vector.tensor_copy(wfb[:, 0:ntot * Hn], wfa[:, 0:ntot * Hn])

            first = True
            gchunk = 0
            for g in range(c.NG):
                G = Gs[g]
                nch = nchs[g]
                for b in range(nch):
                    last = (gchunk == ntot - 1)
                    wf = wfa[:, gchunk * Hn:(gchunk + 1) * Hn]
                    wb = wfb[:, gchunk * Hn:(gchunk + 1) * Hn]
                    eq = eqa[:, gchunk * P:(gchunk + 1) * P]
                    S = sbc.tile([P, Hn * P], BF16, tag="S")
                    eq_b = bass.AP(eq.tensor, eq.offset,
                                   [list(eq.ap[0]), [0, Hn], list(eq.ap[1])])
                    nc.vector.tensor_tensor(
                        out=S[:].rearrange("p (h d) -> p h d", h=Hn),
                        in0=eq_b, in1=wf.to_broadcast([P, Hn, P]),
                        op=AL.mult)
                    BK = 512
                    for h in range(Hn):
                        h_first = (h * Ch) % BK == 0
                        h_last = ((h + 1) * Ch) % BK == 0 or (
                            h == Hn - 1 and not merge_den)
                        nc.tensor.matmul(
                            out=agg[:, h * Ch:(h + 1) * Ch],
                            lhsT=S[:, h * P:(h + 1) * P],
                            rhs=G[:, b * ELEM + h * Ch:b * ELEM + (h + 1) * Ch],
                            start=first and h_first, stop=last and h_last)
                    if merge_den:
                        nc.tensor.matmul(out=den_ap, lhsT=eq,
                                         rhs=wb[:, 0:1], start=False, stop=last)
                    else:
                        nc.tensor.matmul(out=den_ap, lhsT=eq, rhs=wb,
                                         start=first, stop=last)
                    first = False
                    gchunk += 1

            _epilogue(nc, sb, ps1, c, meta, lay, t, agg, den_ap, Hn, Ch, CO,
                      bln_sb, id_sb, fuse, final, y)


def _epilogue(nc, sb, ps1, c, meta, lay, t, agg, den_ap, Hn, Ch, CO,
              bln_sb, id_sb, fuse, final, y):
    denr = sb.tile([P, Hn], F32, tag="denr")
    nc.vector.tensor_scalar(out=denr[:], in0=den_ap, scalar1=1e-16,
                            scalar2=None, op0=AL.add)
    rec = sb.tile([P, Hn], F32, tag="rec")
    nc.vector.reciprocal(rec[:], denr[:])
    o = sb.tile([P, CO], F32, tag="o")
    for h in range(Hn):
        nc.vector.tensor_scalar(
            out=o[:, h * Ch:(h + 1) * Ch], in0=agg[:, h * Ch:(h + 1) * Ch],
            scalar1=rec[:, h:h + 1], scalar2=None, op0=AL.mult)
    ob = sb.tile([P, CO], F32, tag="ob")
    nc.vector.tensor_tensor(out=ob[:], in0=o[:], in1=bln_sb[:, 0:CO], op=AL.add)
    rs = sb.tile([P, 1], F32, tag="rs")
    nc.vector.tensor_reduce(out=rs[:], in_=ob[:], axis=mybir.AxisListType.X,
                            op=AL.add)
    nm = sb.tile([P, 1], F32, tag="nm")
    nc.vector.tensor_scalar(out=nm[:], in0=rs[:], scalar1=-1.0 / CO,
                            scalar2=None, op0=AL.mult)
    xc = sb.tile([P, CO], F32, tag="xc")
    nc.vector.tensor_scalar(out=xc[:], in0=ob[:], scalar1=nm[:, 0:1],
                            scalar2=None, op0=AL.add)
    sq = sb.tile([P, CO], F32, tag="sq")
    vs = sb.tile([P, 1], F32, tag="vs")
    nc.scalar.activation(sq[:], xc[:], AF.Square, accum_out=vs[:])
    vstd = sb.tile([P, 1], F32, tag="vstd")
    nc.vector.tensor_scalar(out=vstd[:], in0=vs[:], scalar1=1.0 / CO,
                            scalar2=LN_EPS, op0=AL.mult, op1=AL.add)
    sd = sb.tile([P, 1], F32, tag="sd")
    nc.scalar.activation(sd[:], vstd[:], AF.Sqrt)
    rstd = sb.tile([P, 1], F32, tag="rstd")
    nc.vector.reciprocal(rstd[:], sd[:])
    y1 = sb.tile([P, CO], F32, tag="y1")
    nc.vector.scalar_tensor_tensor(
        out=y1[:], in0=xc[:], scalar=rstd[:, 0:1],
        in1=bln_sb[:, CO:2 * CO], op0=AL.mult, op1=AL.mult)
    y2 = sb.tile([P, CO], F32, tag="y2")
    nc.vector.tensor_tensor(out=y2[:], in0=y1[:], in1=bln_sb[:, 2 * CO:3 * CO],
                            op=AL.add)

    if final:
        mx = sb.tile([P, 1], F32, tag="mx")
        nc.vector.tensor_reduce(out=mx[:], in_=y2[:],
                                axis=mybir.AxisListType.X, op=AL.max)
        nmx = sb.tile([P, 1], F32, tag="nmx")
        nc.vector.tensor_scalar(out=nmx[:], in0=mx[:], scalar1=-1.0,
                                scalar2=None, op0=AL.mult)
        xs = sb.tile([P, CO], F32, tag="xs")
        nc.vector.tensor_scalar(out=xs[:], in0=y2[:], scalar1=nmx[:, 0:1],
                                scalar2=None, op0=AL.add)
        ex = sb.tile([P, CO], F32, tag="ex")
        se = sb.tile([P, 1], F32, tag="se")
        nc.scalar.activation(ex[:], xs[:], AF.Exp, accum_out=se[:])
        lse = sb.tile([P, 1], F32, tag="lse")
        nc.scalar.activation(lse[:], se[:], AF.Ln)
        nlse = sb.tile([P, 1], F32, tag="nlse")
        nc.vector.tensor_scalar(out=nlse[:], in0=lse[:], scalar1=-1.0,
                                scalar2=None, op0=AL.mult)
        yo = sb.tile([P, CO], F32, tag="yo")
        nc.vector.tensor_scalar(out=yo[:], in0=xs[:], scalar1=nlse[:, 0:1],
                                scalar2=None, op0=AL.add)
        nc.sync.dma_start(y[t * P:(t + 1) * P, :], yo[:])
        return

    x2 = sb.tile([P, CO], BF16, tag="x2")
    nc.vector.scalar_tensor_tensor(
        out=x2[:], in0=y2[:], scalar=NEG_SLOPE_ACT, in1=y2[:],
        op0=AL.mult, op1=AL.max)
    W_sb, KC, Ww = fuse["W_sb"], fuse["KC"], fuse["Ww"]
    CO2, Hn2, ELEMn = fuse["CO"], fuse["Hn2"], fuse["ELEMn"]
    xt2 = sb.tile([P, KC * P], BF16, tag="xt2")
    for k in range(KC):
        scr = ps1.tile([P, P], BF16, tag="scr")
        nc.tensor.transpose(out=scr[:], in_=x2[:, k * P:(k + 1) * P],
                            identity=id_sb[:])
        nc.scalar.copy(xt2[:, k * P:(k + 1) * P], scr[:])
    hp = ps1.tile([P, Ww], F32, tag="hnext")
    _mm_splits(nc, hp, xt2, W_sb, KC, Ww, P)
    _store_hs(nc, sb, hp, CO2, Hn2, ELEMn, fuse["hs_loc"], fuse["d_next"], t)


# --------------------------------------------------------------------------
# entry point
# --------------------------------------------------------------------------

_CACHE = {}


def _get_nc(cfg, meta):
    key = (tuple(sorted((k, str(v)) for k, v in cfg.__dict__.items())),
           tuple(tuple(r) for r in meta.nch))
    if key not in _CACHE:
        _CACHE[key] = build_nc(cfg, meta)
    return _CACHE[key]


def kernel(**inputs):
    inputs = {k: np.asarray(v) for k, v in inputs.items()}
    x = inputs["x"]
    cfg = Cfg(N=x.shape[0], E=inputs["edge_src"].shape[0], F_IN=x.shape[1],
              HEADS=inputs["a_src1"].shape[0], C1=inputs["a_src1"].shape[1],
              C2=inputs["a_src2"].shape[1], NCLS=inputs["W3"].shape[0],
              NCORES=8)
    in_maps, meta = host_prep(cfg, **inputs)
    nc = _get_nc(cfg, meta)
    trace = bool(int(os.environ.get("GAT_TRACE", "0")))
    res = run_bass_kernel_spmd(nc, in_maps, core_ids=list(range(cfg.NCORES)),
                               trace=trace)
    global LAST_EXEC_NS
    LAST_EXEC_NS = res.exec_time_ns
    out = np.concatenate(
        [res.results[cc]["y"][:cfg.NL] for cc in range(cfg.NCORES)], axis=0)
    return out.astype(np.float32)


LAST_EXEC_NS = None


if __name__ == "__main__":
    pass
